# revision 17
# baseline (speedup 1.0000x reference)
"""Trainium2 Bass kernel for fused attention block (QKV proj + RoPE + SDPA + out proj).

Reference computation (B=4, S=2048, HID=2048, H=16, D=128, fp32):
    qkv = hidden @ w_qkv; q,k,v split per head
    q,k = RoPE(q,k, cos,sin)
    attn = softmax(q k^T / sqrt(D)) v          (per batch, head)
    out  = attn.reshape(B,S,H*D) @ w_o

Sharding (8 cores): core c -> (batch b=c//2, head-group g=c%2 of 8 heads).
Each core computes a partial output [S, HID] over its 8 heads; the host sums
the two head-group partials per batch.

v6 over v5 (v5 = 727us; startup was descriptor-throughput-bound: strided
DMAs decomposed into 256B-1KB descriptors that the 16 SDMA engines chew
at ~21ns each, so the first x chunk took ~20us to land):
  - every DMA is now contiguous per partition on BOTH sides: x pre-blocked
    host-side as [NSB,128,NKT,SB] (16KB/partition descriptors), w_q/w_k
    pre-blocked per head chunk (4KB), w_v per 2-head chunk (8KB).
  - phase 1 is single-pass over full S (xt [128,NSB,NKT,SB] = 64KB/part,
    same budget as v5's double-buffered halves): w_q/w_k/w_v stream once
    (12MB instead of 24MB).
  - 36 PE warmup matmuls during the DMA lead-in: PE starts ~0.5us in and
    trips the HAM clock gate to 2.4GHz before the first real matmul.
  - out rows assembled into [128, HID] tiles; one store per 128-row block
    (4KB/partition descriptors, 4x fewer), alternating sync/gpsimd rings.
"""

import os
import sys
import types

sys.path.insert(0, "/opt/trn_rl_repo")

import numpy as np

B, S, HID = 4, 2048, 2048
H, D = 16, 128
HG = 8            # heads per core (head-group)
NCORES = 8
SB = 512          # s-block (matmul free dim)
NSB = S // SB     # 4
NKT = HID // 128  # 16 k-tiles over hidden
NSK = S // 128    # 16 key chunks
SCALE = 1.0 / float(np.sqrt(D))

_STATE = {}
LAST_RESULTS = None


def _ensure_ntff_hook():
    """bass_utils wants antenv.axon_hooks for NTFF tracing under axon; this
    container's antenv lacks it. Register the ctypes-backed hook."""
    try:
        from antenv import axon_hooks  # noqa: F401
        return
    except ImportError:
        pass
    import antenv
    from trn_agent_boot.trn_boot import _ntff_profile_via_ctypes

    mod = types.ModuleType("antenv.axon_hooks")
    _hook = [None]
    mod.set_axon_ntff_profile_hook = lambda h: _hook.__setitem__(0, h)
    mod.get_axon_ntff_profile_hook = lambda: _hook[0]
    sys.modules["antenv.axon_hooks"] = mod
    antenv.axon_hooks = mod
    mod.set_axon_ntff_profile_hook(
        _ntff_profile_via_ctypes("/opt/axon/libaxon_pjrt.so")
    )


def _build():
    import concourse.mybir as mybir
    import concourse.tile as tile
    import concourse.bacc as bacc_mod
    from concourse import bacc

    F32 = mybir.dt.float32
    BF16 = mybir.dt.bfloat16
    EXP = mybir.ActivationFunctionType.Exp

    # Make every activation resolve to the one set that contains Exp AND
    # Copy (natural_log_exp_and_others) so the kernel performs a single
    # ACT table load instead of ping-ponging sets.
    orig_gat = bacc_mod.get_activation_tables

    def _gat_nl_only(arch):
        tabs = orig_gat(arch)
        pref = "natural_log_exp_and_others"
        return {k: (v if k == pref else set()) for k, v in tabs.items()}

    bacc_mod.get_activation_tables = _gat_nl_only
    try:
        nc = bacc.Bacc(None, target_bir_lowering=False, debug=False)

        x_sb = nc.dram_tensor("x_sb", [NSB, 128, NKT, SB], BF16,
                              kind="ExternalInput")
        w_q = nc.dram_tensor("w_q", [HG, 128, NKT, 128], BF16,
                             kind="ExternalInput")
        w_k = nc.dram_tensor("w_k", [HG, 128, NKT, 128], BF16,
                             kind="ExternalInput")
        w_v = nc.dram_tensor("w_v", [4, 128, NKT, 256], BF16,
                             kind="ExternalInput")
        cos_in = nc.dram_tensor("cos_in", [128, S], BF16, kind="ExternalInput")
        sin_in = nc.dram_tensor("sin_in", [128, S], BF16, kind="ExternalInput")
        ones_in = nc.dram_tensor("ones_in", [128, 128], BF16,
                                 kind="ExternalInput")
        w_o = nc.dram_tensor("w_o", [128, HG, HID], BF16, kind="ExternalInput")
        out_p = nc.dram_tensor("out_p", [S, HID], BF16, kind="ExternalOutput")

        with tile.TileContext(nc) as tc:
            with (
                tc.tile_pool(name="persist", bufs=1) as pp,
                tc.tile_pool(name="onesp", bufs=1) as onp,
            ):
                qT = [pp.tile([128, S], BF16, name=f"qT{h}") for h in range(HG)]
                kT = [pp.tile([128, S], BF16, name=f"kT{h}") for h in range(HG)]
                vP = [pp.tile([128, NSK, 256], BF16, name=f"vP{j}")
                      for j in range(HG // 2)]

                ones = onp.tile([128, 128], BF16, tag="ones")
                nc.sync.dma_start(ones[:], ones_in[:])

                # PE warmup: keep the array busy through the DMA lead-in so
                # the HAM clock gate reaches 8/8 before the first real MM.
                with tc.tile_pool(name="wup", bufs=1, space="PSUM") as wup:
                    wps = wup.tile([128, 128], F32, tag="wps")
                    for _ in range(70):
                        nc.tensor.matmul(wps[:], ones[:], ones[:],
                                         start=True, stop=True)

                # ---------------- Phase 1: QKV projection + RoPE ----------
                # The scores PSUM pool (ps_s) doubles as the v-pass PSUM
                # pool and is opened before ps1, so it owns its own banks:
                # phase-2 score matmuls can pre-fill during the q-pass tail
                # instead of waiting for ps1 to drain at the transition.
                ps_s_outer = tc.tile_pool(name="ps_shared", bufs=2,
                                          space="PSUM")
                ps_s = ps_s_outer.__enter__()
                with (
                    tc.tile_pool(name="p1cs", bufs=1) as csp,
                    tc.tile_pool(name="p1xt", bufs=1) as xtp,
                    tc.tile_pool(name="p1w", bufs=2) as wp,
                    tc.tile_pool(name="p1vw", bufs=2) as vwp,
                    tc.tile_pool(name="p1ev", bufs=3) as evp,
                ):
                    xt = xtp.tile([128, NSB, NKT, SB], BF16, tag="xt")
                    # Everything rides the sync ring in need-time order.  The
                    # v-pass opens phase 1 because its x first-touch rate
                    # (~287GB/s) fits under HBM bandwidth, unlike the qk
                    # passes (~578GB/s) - so x streams in with no PE stalls.
                    nc.sync.dma_start(xt[:, 0, :, :], x_sb[0, :, :, :])
                    wv_first = vwp.tile([128, NKT, 256], BF16, tag="wv",
                                        name="wv_first")
                    nc.sync.dma_start(wv_first[:], w_v[0, :, :, :])
                    for sb in range(1, NSB):
                        nc.sync.dma_start(xt[:, sb, :, :], x_sb[sb, :, :, :])
                    cosf = csp.tile([128, S], BF16, tag="cos")
                    sinf = csp.tile([128, S], BF16, tag="sin")

                    def qk_pass(w_dram, dest, opener=False, hooks=None):
                        # An opener pass first touches only x chunk 0 (sb=0)
                        # for every head, giving the PE ~28us of work while
                        # the remaining x chunks load; hooks emit those chunk
                        # DMAs at the right sync-ring FIFO positions.
                        if opener:
                            rounds = [[0]] * HG + [[1, 2, 3]] * HG
                        else:
                            rounds = [[0, 1, 2, 3]] * HG
                        for i, sbs in enumerate(rounds):
                            c = i % HG
                            wc = wp.tile([128, NKT, 128], BF16, tag="w")
                            weng = nc.scalar if (opener and i == 0) else nc.sync
                            weng.dma_start(wc[:], w_dram[c, :, :, :])
                            for fn in (hooks or {}).get(i, ()):
                                fn()
                            for sb in sbs:
                                gsl = slice(sb * SB, (sb + 1) * SB)
                                ps = ps1.tile([128, SB], F32, tag="ps_qk")
                                for kt in range(NKT):
                                    nc.tensor.matmul(
                                        ps[:],
                                        wc[:, kt, :],
                                        xt[:, sb, kt, :],
                                        start=(kt == 0),
                                        stop=(kt == NKT - 1),
                                    )
                                qt = evp.tile([128, SB], BF16, tag="qt")
                                nc.scalar.copy(qt[:], ps[:])
                                qs = evp.tile([128, SB], BF16, tag="qs")
                                nc.gpsimd.dma_start(qs[0:64, :], qt[64:128, :])
                                nc.gpsimd.dma_start(qs[64:128, :], qt[0:64, :])
                                nc.vector.tensor_mul(qt[:], qt[:], cosf[:, gsl])
                                nc.vector.tensor_mul(qs[:], qs[:], sinf[:, gsl])
                                nc.vector.tensor_add(
                                    dest[c][:, gsl], qt[:], qs[:]
                                )

                    def v_pass():
                        for vc in range(4):
                            if vc == 0:
                                wvc = wv_first
                            else:
                                wvc = vwp.tile([128, NKT, 256], BF16,
                                               tag="wv")
                                nc.sync.dma_start(wvc[:], w_v[vc, :, :, :])
                            for ss in range(NSK):
                                sb, j = divmod(ss, 4)
                                ps = ps_s.tile([128, 2 * SB], F32,
                                               tag="ps_s")
                                for kt in range(NKT):
                                    nc.tensor.matmul(
                                        ps[:, 0:256],
                                        xt[:, sb, kt,
                                           j * 128 : (j + 1) * 128],
                                        wvc[:, kt, :],
                                        start=(kt == 0),
                                        stop=(kt == NKT - 1),
                                    )
                                nc.vector.tensor_copy(vP[vc][:, ss, :],
                                                      ps[:, 0:256])

                    with tc.tile_pool(name="p1ps", bufs=4,
                                      space="PSUM") as ps1:
                        v_pass()
                        nc.sync.dma_start(cosf[:], cos_in[:])
                        nc.sync.dma_start(sinf[:], sin_in[:])
                        qk_pass(w_k, kT)
                        qk_pass(w_q, qT)

                # ---------------- Phases 2+3 (fused, software-pipelined) --
                with (
                    tc.tile_pool(name="p2o", bufs=1) as otp,
                    tc.tile_pool(name="p3w", bufs=1) as wop,
                    tc.tile_pool(name="p2pt", bufs=10) as ptp,
                    tc.tile_pool(name="p2red", bufs=8) as rdp,
                    tc.tile_pool(name="p2r", bufs=1) as rp,
                    tc.tile_pool(name="p3o", bufs=4) as outp,
                    tc.tile_pool(name="p2ps_o", bufs=2, space="PSUM") as ps_o,
                    tc.tile_pool(name="p2ps_m", bufs=2, space="PSUM") as ps_m,
                ):
                    oT = [otp.tile([128, S], BF16, name=f"oT{h}")
                          for h in range(HG)]
                    wo = wop.tile([128, HG, HID], BF16, tag="wo")
                    nc.sync.dma_start(wo[:], w_o[:])

                    def emit_scores_exp_tree(h, sqb):
                        qsl = slice(sqb * SB, (sqb + 1) * SB)
                        pts = []
                        for sk2 in range(NSK // 2):
                            pss = ps_s.tile([128, 2 * SB], F32, tag="ps_s")
                            for j in range(2):
                                sk = 2 * sk2 + j
                                nc.tensor.matmul(
                                    pss[:, j * SB : (j + 1) * SB],
                                    kT[h][:, sk * 128 : (sk + 1) * 128],
                                    qT[h][:, qsl],
                                    start=True,
                                    stop=True,
                                )
                            pt = ptp.tile([128, 2 * SB], BF16, tag="pt")
                            nc.scalar.activation(pt[:], pss[:], EXP,
                                                 scale=SCALE)
                            pts.append(pt)
                        lvl = []
                        for i in range(8):
                            a = rdp.tile([128, SB], BF16, tag="ra")
                            nc.vector.tensor_add(
                                a[:], pts[i][:, 0:SB], pts[i][:, SB:2 * SB]
                            )
                            lvl.append(a)
                        while len(lvl) > 1:
                            nxt = []
                            for i in range(0, len(lvl), 2):
                                b = rdp.tile([128, SB], BF16, tag="rb")
                                nc.vector.tensor_add(b[:], lvl[i][:],
                                                     lvl[i + 1][:])
                                nxt.append(b)
                            lvl = nxt
                        return pts, lvl[0]

                    def emit_finish(h, sqb, pts, pacc):
                        qsl = slice(sqb * SB, (sqb + 1) * SB)
                        vp = vP[h // 2]
                        voff = (h % 2) * 128
                        pso = ps_o.tile([128, SB], F32, tag="ps_o")
                        for sk in range(NSK):
                            nc.tensor.matmul(
                                pso[:],
                                vp[:, sk, voff : voff + 128],
                                pts[sk // 2][:, (sk % 2) * SB
                                             : (sk % 2 + 1) * SB],
                                start=(sk == 0), stop=(sk == NSK - 1),
                            )
                        psd = ps_m.tile([128, SB], F32, tag="m")
                        nc.tensor.matmul(psd[:], ones[:], pacc[:],
                                         start=True, stop=True)
                        rec = rp.tile([128, SB], F32, tag="rec")
                        nc.vector.reciprocal_approx_fast(rec[:], psd[:])
                        nc.vector.tensor_mul(oT[h][:, qsl], pso[:], rec[:])

                    store_rings = [nc.sync, nc.gpsimd, nc.scalar]

                    def emit_p3(idx, sc, nb):
                        ssl = slice(sc * 128, (sc + 1) * 128)
                        ps = ps_m.tile([128, SB], F32, tag="m")
                        for h2 in range(HG):
                            nc.tensor.matmul(
                                ps[:],
                                oT[h2][:, ssl],
                                wo[:, h2, nb * SB : (nb + 1) * SB],
                                start=(h2 == 0),
                                stop=(h2 == HG - 1),
                            )
                        ot = outp.tile([128, SB], BF16, tag="out")
                        nc.vector.tensor_copy(ot[:], ps[:])
                        store_rings[idx % 3].dma_start(
                            out_p[ssl, nb * SB : (nb + 1) * SB], ot[:]
                        )

                    p3_idx = 0
                    prev = None
                    p3_pending = []
                    for sqb in range(NSB):
                        for h in range(HG):
                            cur = (h, sqb) + emit_scores_exp_tree(h, sqb)
                            if prev is not None:
                                emit_finish(*prev)
                                if prev[0] == HG - 1:
                                    r = prev[1]
                                    p3_pending.extend(
                                        (4 * r + i, nb)
                                        for i in range(4)
                                        for nb in range(HID // SB)
                                    )
                            for _ in range(2):
                                if p3_pending:
                                    emit_p3(p3_idx, *p3_pending.pop(0))
                                    p3_idx += 1
                            prev = cur
                    emit_finish(*prev)
                    p3_pending.extend(
                        (4 * (NSB - 1) + i, nb)
                        for i in range(4)
                        for nb in range(HID // SB)
                    )
                    for args in p3_pending:
                        emit_p3(p3_idx, *args)
                        p3_idx += 1

                ps_s_outer.__exit__(None, None, None)

        nc.compile()
    finally:
        bacc_mod.get_activation_tables = orig_gat
    return nc


def _get_nc():
    if "nc" not in _STATE:
        _STATE["nc"] = _build()
    return _STATE["nc"]


def kernel(hidden_states, cos, sin, w_qkv, w_o):
    global LAST_RESULTS
    from concourse.bass_utils import run_bass_kernel_spmd
    import ml_dtypes

    bf16 = ml_dtypes.bfloat16

    trace = os.environ.get("KERNEL_TRACE", "") == "1"
    if trace:
        _ensure_ntff_hook()

    hidden_states = np.asarray(hidden_states, dtype=np.float32)
    cos = np.asarray(cos, dtype=np.float32)
    sin = np.asarray(sin, dtype=np.float32)
    w_qkv = np.asarray(w_qkv, dtype=np.float32)
    w_o = np.asarray(w_o, dtype=np.float32)

    cos_t = np.ascontiguousarray(cos.T)                      # [128, S]
    sin_t = np.ascontiguousarray(sin.T)
    sin_rot = np.concatenate([-sin_t[:64], sin_t[64:]], axis=0)
    ones = np.ones((128, 128), np.float32)

    def ktile(w):  # [HID, N] -> [128, NKT, N]
        n = w.shape[1]
        return np.ascontiguousarray(w.reshape(NKT, 128, n).transpose(1, 0, 2))

    def cblock(w, csz):  # [128, NKT, N] -> [N//csz, 128, NKT, csz]
        n = w.shape[2]
        return np.ascontiguousarray(
            w.reshape(128, NKT, n // csz, csz).transpose(2, 0, 1, 3)
        )

    in_maps = []
    for c in range(NCORES):
        b, g = divmod(c, 2)
        cs = slice(g * HG * D, (g + 1) * HG * D)
        wq = cblock(ktile(w_qkv[:, 0:H * D][:, cs]), 128)
        wk = cblock(ktile(w_qkv[:, H * D:2 * H * D][:, cs]), 128)
        wv = cblock(ktile(w_qkv[:, 2 * H * D:3 * H * D][:, cs]), 256)
        wo_c = w_o[cs, :]
        wo_r = np.ascontiguousarray(
            wo_c.reshape(HG, 128, HID).transpose(1, 0, 2)
        )
        # x^T k-tiled and s-blocked: [NSB, 128, NKT, SB]
        # x_sb[sb, p, kt, s] = x[sb*SB+s, kt*128+p]
        xt = hidden_states[b].T.reshape(NKT, 128, NSB, SB)
        xt = np.ascontiguousarray(xt.transpose(2, 1, 0, 3))
        in_maps.append({
            "x_sb": xt.astype(bf16),
            "w_q": wq.astype(bf16), "w_k": wk.astype(bf16),
            "w_v": wv.astype(bf16),
            "cos_in": cos_t.astype(bf16), "sin_in": sin_rot.astype(bf16),
            "ones_in": ones.astype(bf16),
            "w_o": wo_r.astype(bf16),
        })

    nc = _get_nc()
    res = run_bass_kernel_spmd(
        nc, in_maps, core_ids=list(range(NCORES)), trace=trace
    )
    LAST_RESULTS = res

    out = np.empty((B, S, HID), np.float32)
    for b in range(B):
        out[b] = (res.results[2 * b]["out_p"].astype(np.float32)
                  + res.results[2 * b + 1]["out_p"].astype(np.float32))
    return out


# revision 18
# speedup vs baseline: 1.0131x; 1.0131x over previous
"""Trainium2 Bass kernel for fused attention block (QKV proj + RoPE + SDPA + out proj).

Reference computation (B=4, S=2048, HID=2048, H=16, D=128, fp32):
    qkv = hidden @ w_qkv; q,k,v split per head
    q,k = RoPE(q,k, cos,sin)
    attn = softmax(q k^T / sqrt(D)) v          (per batch, head)
    out  = attn.reshape(B,S,H*D) @ w_o

Sharding (8 cores): core c -> (batch b=c//2, head-group g=c%2 of 8 heads).
Each core computes a partial output [S, HID] over its 8 heads; the host sums
the two head-group partials per batch.

v6 over v5 (v5 = 727us; startup was descriptor-throughput-bound: strided
DMAs decomposed into 256B-1KB descriptors that the 16 SDMA engines chew
at ~21ns each, so the first x chunk took ~20us to land):
  - every DMA is now contiguous per partition on BOTH sides: x pre-blocked
    host-side as [NSB,128,NKT,SB] (16KB/partition descriptors), w_q/w_k
    pre-blocked per head chunk (4KB), w_v per 2-head chunk (8KB).
  - phase 1 is single-pass over full S (xt [128,NSB,NKT,SB] = 64KB/part,
    same budget as v5's double-buffered halves): w_q/w_k/w_v stream once
    (12MB instead of 24MB).
  - 36 PE warmup matmuls during the DMA lead-in: PE starts ~0.5us in and
    trips the HAM clock gate to 2.4GHz before the first real matmul.
  - out rows assembled into [128, HID] tiles; one store per 128-row block
    (4KB/partition descriptors, 4x fewer), alternating sync/gpsimd rings.
"""

import os
import sys
import types

sys.path.insert(0, "/opt/trn_rl_repo")

import numpy as np

B, S, HID = 4, 2048, 2048
H, D = 16, 128
HG = 8            # heads per core (head-group)
NCORES = 8
SB = 512          # s-block (matmul free dim)
NSB = S // SB     # 4
NKT = HID // 128  # 16 k-tiles over hidden
NSK = S // 128    # 16 key chunks
SCALE = 1.0 / float(np.sqrt(D))

_STATE = {}
LAST_RESULTS = None


def _ensure_ntff_hook():
    """bass_utils wants antenv.axon_hooks for NTFF tracing under axon; this
    container's antenv lacks it. Register the ctypes-backed hook."""
    try:
        from antenv import axon_hooks  # noqa: F401
        return
    except ImportError:
        pass
    import antenv
    from trn_agent_boot.trn_boot import _ntff_profile_via_ctypes

    mod = types.ModuleType("antenv.axon_hooks")
    _hook = [None]
    mod.set_axon_ntff_profile_hook = lambda h: _hook.__setitem__(0, h)
    mod.get_axon_ntff_profile_hook = lambda: _hook[0]
    sys.modules["antenv.axon_hooks"] = mod
    antenv.axon_hooks = mod
    mod.set_axon_ntff_profile_hook(
        _ntff_profile_via_ctypes("/opt/axon/libaxon_pjrt.so")
    )


def _build():
    import concourse.mybir as mybir
    import concourse.tile as tile
    import concourse.bacc as bacc_mod
    from concourse import bacc

    F32 = mybir.dt.float32
    BF16 = mybir.dt.bfloat16
    EXP = mybir.ActivationFunctionType.Exp

    # Make every activation resolve to the one set that contains Exp AND
    # Copy (natural_log_exp_and_others) so the kernel performs a single
    # ACT table load instead of ping-ponging sets.
    orig_gat = bacc_mod.get_activation_tables

    def _gat_nl_only(arch):
        tabs = orig_gat(arch)
        pref = "natural_log_exp_and_others"
        return {k: (v if k == pref else set()) for k, v in tabs.items()}

    bacc_mod.get_activation_tables = _gat_nl_only
    try:
        nc = bacc.Bacc(None, target_bir_lowering=False, debug=False)

        x_sb = nc.dram_tensor("x_sb", [NSB, 128, NKT, SB], BF16,
                              kind="ExternalInput")
        w_q = nc.dram_tensor("w_q", [HG, 128, NKT, 128], BF16,
                             kind="ExternalInput")
        w_k = nc.dram_tensor("w_k", [HG, 128, NKT, 128], BF16,
                             kind="ExternalInput")
        w_v = nc.dram_tensor("w_v", [4, 128, NKT, 256], BF16,
                             kind="ExternalInput")
        cos_in = nc.dram_tensor("cos_in", [128, S], BF16, kind="ExternalInput")
        sin_in = nc.dram_tensor("sin_in", [128, S], BF16, kind="ExternalInput")
        ones_in = nc.dram_tensor("ones_in", [128, 128], BF16,
                                 kind="ExternalInput")
        w_o = nc.dram_tensor("w_o", [128, HG, HID], BF16, kind="ExternalInput")
        out_p = nc.dram_tensor("out_p", [S, HID], BF16, kind="ExternalOutput")

        with tile.TileContext(nc) as tc:
            with (
                tc.tile_pool(name="persist", bufs=1) as pp,
                tc.tile_pool(name="onesp", bufs=1) as onp,
            ):
                qT = [pp.tile([128, S], BF16, name=f"qT{h}") for h in range(HG)]
                kT = [pp.tile([128, S], BF16, name=f"kT{h}") for h in range(HG)]
                vP = [pp.tile([128, NSK, 256], BF16, name=f"vP{j}")
                      for j in range(HG // 2)]

                ones = onp.tile([128, 128], BF16, tag="ones")
                nc.sync.dma_start(ones[:], ones_in[:])

                # PE warmup: keep the array busy through the DMA lead-in so
                # the HAM clock gate reaches 8/8 before the first real MM.
                with tc.tile_pool(name="wup", bufs=1, space="PSUM") as wup:
                    wps = wup.tile([128, 128], F32, tag="wps")
                    for _ in range(70):
                        nc.tensor.matmul(wps[:], ones[:], ones[:],
                                         start=True, stop=True)

                # ---------------- Phase 1: QKV projection + RoPE ----------
                # Order: v(vc0) opens (its x first-touch rate ~287GB/s
                # fits under HBM, so x streams in stall-free), q/k follow,
                # v(vc1-3) closes phase 1 - giving an ~80us window where the
                # qk-side pools (csp/wp/evp) are idle so phase-2 pt/score
                # work can pre-fill before the transition.
                with (
                    tc.tile_pool(name="p1cs", bufs=1) as csp,
                    tc.tile_pool(name="p1xt", bufs=1) as xtp,
                    tc.tile_pool(name="p1w", bufs=2) as wp,
                    tc.tile_pool(name="p1vw", bufs=2) as vwp,
                    tc.tile_pool(name="p1ev", bufs=3) as evp,
                ):
                    xt = xtp.tile([128, NSB, NKT, SB], BF16, tag="xt")
                    # Everything rides the sync ring in need-time order.  The
                    # v-pass opens phase 1 because its x first-touch rate
                    # (~287GB/s) fits under HBM bandwidth, unlike the qk
                    # passes (~578GB/s) - so x streams in with no PE stalls.
                    nc.sync.dma_start(xt[:, 0, :, :], x_sb[0, :, :, :])
                    wv_first = vwp.tile([128, NKT, 256], BF16, tag="wv",
                                        name="wv_first")
                    nc.sync.dma_start(wv_first[:], w_v[0, :, :, :])
                    for sb in range(1, NSB):
                        nc.sync.dma_start(xt[:, sb, :, :], x_sb[sb, :, :, :])
                    cosf = csp.tile([128, S], BF16, tag="cos")
                    sinf = csp.tile([128, S], BF16, tag="sin")

                    def qk_pass(w_dram, dest, opener=False, hooks=None):
                        # An opener pass first touches only x chunk 0 (sb=0)
                        # for every head, giving the PE ~28us of work while
                        # the remaining x chunks load; hooks emit those chunk
                        # DMAs at the right sync-ring FIFO positions.
                        if opener:
                            rounds = [[0]] * HG + [[1, 2, 3]] * HG
                        else:
                            rounds = [[0, 1, 2, 3]] * HG
                        for i, sbs in enumerate(rounds):
                            c = i % HG
                            wc = wp.tile([128, NKT, 128], BF16, tag="w")
                            weng = nc.scalar if (opener and i == 0) else nc.sync
                            weng.dma_start(wc[:], w_dram[c, :, :, :])
                            for fn in (hooks or {}).get(i, ()):
                                fn()
                            for sb in sbs:
                                gsl = slice(sb * SB, (sb + 1) * SB)
                                ps = ps1.tile([128, SB], F32, tag="ps_qk")
                                for kt in range(NKT):
                                    nc.tensor.matmul(
                                        ps[:],
                                        wc[:, kt, :],
                                        xt[:, sb, kt, :],
                                        start=(kt == 0),
                                        stop=(kt == NKT - 1),
                                    )
                                qt = evp.tile([128, SB], BF16, tag="qt")
                                nc.scalar.copy(qt[:], ps[:])
                                qs = evp.tile([128, SB], BF16, tag="qs")
                                nc.gpsimd.dma_start(qs[0:64, :], qt[64:128, :])
                                nc.gpsimd.dma_start(qs[64:128, :], qt[0:64, :])
                                nc.vector.tensor_mul(qt[:], qt[:], cosf[:, gsl])
                                nc.vector.tensor_mul(qs[:], qs[:], sinf[:, gsl])
                                nc.vector.tensor_add(
                                    dest[c][:, gsl], qt[:], qs[:]
                                )

                    def v_pass(vcs):
                        for vc in vcs:
                            if vc == 0:
                                wvc = wv_first
                            else:
                                wvc = vwp.tile([128, NKT, 256], BF16,
                                               tag="wv")
                                nc.sync.dma_start(wvc[:], w_v[vc, :, :, :])
                            for ss in range(NSK):
                                sb, j = divmod(ss, 4)
                                ps = vps.tile([128, 256], F32, tag="ps_v")
                                for kt in range(NKT):
                                    nc.tensor.matmul(
                                        ps[:],
                                        xt[:, sb, kt,
                                           j * 128 : (j + 1) * 128],
                                        wvc[:, kt, :],
                                        start=(kt == 0),
                                        stop=(kt == NKT - 1),
                                    )
                                nc.vector.tensor_copy(vP[vc][:, ss, :], ps[:])

                    with (
                        tc.tile_pool(name="p1ps", bufs=4,
                                     space="PSUM") as ps1,
                        tc.tile_pool(name="p1vps", bufs=2,
                                     space="PSUM") as vps,
                    ):
                        v_pass([0])
                        nc.sync.dma_start(cosf[:], cos_in[:])
                        nc.sync.dma_start(sinf[:], sin_in[:])
                        qk_pass(w_q, qT)
                        qk_pass(w_k, kT)
                        v_pass([1, 2, 3])

                # ---------------- Phases 2+3 (fused, software-pipelined) --
                with (
                    tc.tile_pool(name="p2o", bufs=1) as otp,
                    tc.tile_pool(name="p3w", bufs=1) as wop,
                    tc.tile_pool(name="p2pt", bufs=10) as ptp,
                    tc.tile_pool(name="p2red", bufs=8) as rdp,
                    tc.tile_pool(name="p2r", bufs=1) as rp,
                    tc.tile_pool(name="p3o", bufs=4) as outp,
                    tc.tile_pool(name="p2ps_s", bufs=2, space="PSUM") as ps_s,
                    tc.tile_pool(name="p2ps_o", bufs=2, space="PSUM") as ps_o,
                    tc.tile_pool(name="p2ps_m", bufs=2, space="PSUM") as ps_m,
                ):
                    oT = [otp.tile([128, S], BF16, name=f"oT{h}")
                          for h in range(HG)]
                    wo = wop.tile([128, HG, HID], BF16, tag="wo")
                    nc.sync.dma_start(wo[:], w_o[:])

                    def emit_scores_exp_tree(h, sqb):
                        qsl = slice(sqb * SB, (sqb + 1) * SB)
                        pts = []
                        for sk2 in range(NSK // 2):
                            pss = ps_s.tile([128, 2 * SB], F32, tag="ps_s")
                            for j in range(2):
                                sk = 2 * sk2 + j
                                nc.tensor.matmul(
                                    pss[:, j * SB : (j + 1) * SB],
                                    kT[h][:, sk * 128 : (sk + 1) * 128],
                                    qT[h][:, qsl],
                                    start=True,
                                    stop=True,
                                )
                            pt = ptp.tile([128, 2 * SB], BF16, tag="pt")
                            nc.scalar.activation(pt[:], pss[:], EXP,
                                                 scale=SCALE)
                            pts.append(pt)
                        lvl = []
                        for i in range(8):
                            a = rdp.tile([128, SB], BF16, tag="ra")
                            nc.vector.tensor_add(
                                a[:], pts[i][:, 0:SB], pts[i][:, SB:2 * SB]
                            )
                            lvl.append(a)
                        while len(lvl) > 1:
                            nxt = []
                            for i in range(0, len(lvl), 2):
                                b = rdp.tile([128, SB], BF16, tag="rb")
                                nc.vector.tensor_add(b[:], lvl[i][:],
                                                     lvl[i + 1][:])
                                nxt.append(b)
                            lvl = nxt
                        return pts, lvl[0]

                    def emit_finish(h, sqb, pts, pacc):
                        qsl = slice(sqb * SB, (sqb + 1) * SB)
                        vp = vP[h // 2]
                        voff = (h % 2) * 128
                        pso = ps_o.tile([128, SB], F32, tag="ps_o")
                        for sk in range(NSK):
                            nc.tensor.matmul(
                                pso[:],
                                vp[:, sk, voff : voff + 128],
                                pts[sk // 2][:, (sk % 2) * SB
                                             : (sk % 2 + 1) * SB],
                                start=(sk == 0), stop=(sk == NSK - 1),
                            )
                        psd = ps_m.tile([128, SB], F32, tag="m")
                        nc.tensor.matmul(psd[:], ones[:], pacc[:],
                                         start=True, stop=True)
                        rec = rp.tile([128, SB], F32, tag="rec")
                        nc.vector.reciprocal_approx_fast(rec[:], psd[:])
                        nc.vector.tensor_mul(oT[h][:, qsl], pso[:], rec[:])

                    store_rings = [nc.sync, nc.gpsimd, nc.scalar]

                    def emit_p3(idx, sc, nb):
                        ssl = slice(sc * 128, (sc + 1) * 128)
                        ps = ps_m.tile([128, SB], F32, tag="m")
                        for h2 in range(HG):
                            nc.tensor.matmul(
                                ps[:],
                                oT[h2][:, ssl],
                                wo[:, h2, nb * SB : (nb + 1) * SB],
                                start=(h2 == 0),
                                stop=(h2 == HG - 1),
                            )
                        ot = outp.tile([128, SB], BF16, tag="out")
                        nc.vector.tensor_copy(ot[:], ps[:])
                        store_rings[idx % 3].dma_start(
                            out_p[ssl, nb * SB : (nb + 1) * SB], ot[:]
                        )

                    p3_idx = 0
                    prev = None
                    p3_pending = []
                    for sqb in range(NSB):
                        for h in range(HG):
                            cur = (h, sqb) + emit_scores_exp_tree(h, sqb)
                            if prev is not None:
                                emit_finish(*prev)
                                if prev[0] == HG - 1:
                                    r = prev[1]
                                    p3_pending.extend(
                                        (4 * r + i, nb)
                                        for i in range(4)
                                        for nb in range(HID // SB)
                                    )
                            for _ in range(2):
                                if p3_pending:
                                    emit_p3(p3_idx, *p3_pending.pop(0))
                                    p3_idx += 1
                            prev = cur
                    emit_finish(*prev)
                    p3_pending.extend(
                        (4 * (NSB - 1) + i, nb)
                        for i in range(4)
                        for nb in range(HID // SB)
                    )
                    for args in p3_pending:
                        emit_p3(p3_idx, *args)
                        p3_idx += 1

        nc.compile()
    finally:
        bacc_mod.get_activation_tables = orig_gat
    return nc


def _get_nc():
    if "nc" not in _STATE:
        _STATE["nc"] = _build()
    return _STATE["nc"]


def kernel(hidden_states, cos, sin, w_qkv, w_o):
    global LAST_RESULTS
    from concourse.bass_utils import run_bass_kernel_spmd
    import ml_dtypes

    bf16 = ml_dtypes.bfloat16

    trace = os.environ.get("KERNEL_TRACE", "") == "1"
    if trace:
        _ensure_ntff_hook()

    hidden_states = np.asarray(hidden_states, dtype=np.float32)
    cos = np.asarray(cos, dtype=np.float32)
    sin = np.asarray(sin, dtype=np.float32)
    w_qkv = np.asarray(w_qkv, dtype=np.float32)
    w_o = np.asarray(w_o, dtype=np.float32)

    cos_t = np.ascontiguousarray(cos.T)                      # [128, S]
    sin_t = np.ascontiguousarray(sin.T)
    sin_rot = np.concatenate([-sin_t[:64], sin_t[64:]], axis=0)
    ones = np.ones((128, 128), np.float32)

    def ktile(w):  # [HID, N] -> [128, NKT, N]
        n = w.shape[1]
        return np.ascontiguousarray(w.reshape(NKT, 128, n).transpose(1, 0, 2))

    def cblock(w, csz):  # [128, NKT, N] -> [N//csz, 128, NKT, csz]
        n = w.shape[2]
        return np.ascontiguousarray(
            w.reshape(128, NKT, n // csz, csz).transpose(2, 0, 1, 3)
        )

    in_maps = []
    for c in range(NCORES):
        b, g = divmod(c, 2)
        cs = slice(g * HG * D, (g + 1) * HG * D)
        wq = cblock(ktile(w_qkv[:, 0:H * D][:, cs]), 128)
        wk = cblock(ktile(w_qkv[:, H * D:2 * H * D][:, cs]), 128)
        wv = cblock(ktile(w_qkv[:, 2 * H * D:3 * H * D][:, cs]), 256)
        wo_c = w_o[cs, :]
        wo_r = np.ascontiguousarray(
            wo_c.reshape(HG, 128, HID).transpose(1, 0, 2)
        )
        # x^T k-tiled and s-blocked: [NSB, 128, NKT, SB]
        # x_sb[sb, p, kt, s] = x[sb*SB+s, kt*128+p]
        xt = hidden_states[b].T.reshape(NKT, 128, NSB, SB)
        xt = np.ascontiguousarray(xt.transpose(2, 1, 0, 3))
        in_maps.append({
            "x_sb": xt.astype(bf16),
            "w_q": wq.astype(bf16), "w_k": wk.astype(bf16),
            "w_v": wv.astype(bf16),
            "cos_in": cos_t.astype(bf16), "sin_in": sin_rot.astype(bf16),
            "ones_in": ones.astype(bf16),
            "w_o": wo_r.astype(bf16),
        })

    nc = _get_nc()
    res = run_bass_kernel_spmd(
        nc, in_maps, core_ids=list(range(NCORES)), trace=trace
    )
    LAST_RESULTS = res

    out = np.empty((B, S, HID), np.float32)
    for b in range(B):
        out[b] = (res.results[2 * b]["out_p"].astype(np.float32)
                  + res.results[2 * b + 1]["out_p"].astype(np.float32))
    return out


# revision 19
# speedup vs baseline: 1.0154x; 1.0022x over previous
"""Trainium2 Bass kernel for fused attention block (QKV proj + RoPE + SDPA + out proj).

Reference computation (B=4, S=2048, HID=2048, H=16, D=128, fp32):
    qkv = hidden @ w_qkv; q,k,v split per head
    q,k = RoPE(q,k, cos,sin)
    attn = softmax(q k^T / sqrt(D)) v          (per batch, head)
    out  = attn.reshape(B,S,H*D) @ w_o

Sharding (8 cores): core c -> (batch b=c//2, head-group g=c%2 of 8 heads).
Each core computes a partial output [S, HID] over its 8 heads; the host sums
the two head-group partials per batch.

v6 over v5 (v5 = 727us; startup was descriptor-throughput-bound: strided
DMAs decomposed into 256B-1KB descriptors that the 16 SDMA engines chew
at ~21ns each, so the first x chunk took ~20us to land):
  - every DMA is now contiguous per partition on BOTH sides: x pre-blocked
    host-side as [NSB,128,NKT,SB] (16KB/partition descriptors), w_q/w_k
    pre-blocked per head chunk (4KB), w_v per 2-head chunk (8KB).
  - phase 1 is single-pass over full S (xt [128,NSB,NKT,SB] = 64KB/part,
    same budget as v5's double-buffered halves): w_q/w_k/w_v stream once
    (12MB instead of 24MB).
  - 36 PE warmup matmuls during the DMA lead-in: PE starts ~0.5us in and
    trips the HAM clock gate to 2.4GHz before the first real matmul.
  - out rows assembled into [128, HID] tiles; one store per 128-row block
    (4KB/partition descriptors, 4x fewer), alternating sync/gpsimd rings.
"""

import os
import sys
import types

sys.path.insert(0, "/opt/trn_rl_repo")

import numpy as np

B, S, HID = 4, 2048, 2048
H, D = 16, 128
HG = 8            # heads per core (head-group)
NCORES = 8
SB = 512          # s-block (matmul free dim)
NSB = S // SB     # 4
NKT = HID // 128  # 16 k-tiles over hidden
NSK = S // 128    # 16 key chunks
SCALE = 1.0 / float(np.sqrt(D))

_STATE = {}
LAST_RESULTS = None


def _ensure_ntff_hook():
    """bass_utils wants antenv.axon_hooks for NTFF tracing under axon; this
    container's antenv lacks it. Register the ctypes-backed hook."""
    try:
        from antenv import axon_hooks  # noqa: F401
        return
    except ImportError:
        pass
    import antenv
    from trn_agent_boot.trn_boot import _ntff_profile_via_ctypes

    mod = types.ModuleType("antenv.axon_hooks")
    _hook = [None]
    mod.set_axon_ntff_profile_hook = lambda h: _hook.__setitem__(0, h)
    mod.get_axon_ntff_profile_hook = lambda: _hook[0]
    sys.modules["antenv.axon_hooks"] = mod
    antenv.axon_hooks = mod
    mod.set_axon_ntff_profile_hook(
        _ntff_profile_via_ctypes("/opt/axon/libaxon_pjrt.so")
    )


def _build():
    import concourse.mybir as mybir
    import concourse.tile as tile
    import concourse.bacc as bacc_mod
    from concourse import bacc

    F32 = mybir.dt.float32
    BF16 = mybir.dt.bfloat16
    EXP = mybir.ActivationFunctionType.Exp

    # Make every activation resolve to the one set that contains Exp AND
    # Copy (natural_log_exp_and_others) so the kernel performs a single
    # ACT table load instead of ping-ponging sets.
    orig_gat = bacc_mod.get_activation_tables

    def _gat_nl_only(arch):
        tabs = orig_gat(arch)
        pref = "natural_log_exp_and_others"
        return {k: (v if k == pref else set()) for k, v in tabs.items()}

    bacc_mod.get_activation_tables = _gat_nl_only
    try:
        nc = bacc.Bacc(None, target_bir_lowering=False, debug=False)

        x_sb = nc.dram_tensor("x_sb", [NSB, 128, NKT, SB], BF16,
                              kind="ExternalInput")
        w_q = nc.dram_tensor("w_q", [HG, 128, NKT, 128], BF16,
                             kind="ExternalInput")
        w_k = nc.dram_tensor("w_k", [HG, 128, NKT, 128], BF16,
                             kind="ExternalInput")
        w_v = nc.dram_tensor("w_v", [4, 128, NKT, 256], BF16,
                             kind="ExternalInput")
        cos_in = nc.dram_tensor("cos_in", [128, S], BF16, kind="ExternalInput")
        sin_in = nc.dram_tensor("sin_in", [128, S], BF16, kind="ExternalInput")
        w_o = nc.dram_tensor("w_o", [128, HG, HID], BF16, kind="ExternalInput")
        out_p = nc.dram_tensor("out_p", [S, HID], BF16, kind="ExternalOutput")

        with tile.TileContext(nc) as tc:
            with (
                tc.tile_pool(name="persist", bufs=1) as pp,
                tc.tile_pool(name="onesp", bufs=1) as onp,
            ):
                qT = [pp.tile([128, S], BF16, name=f"qT{h}") for h in range(HG)]
                kT = [pp.tile([128, S], BF16, name=f"kT{h}") for h in range(HG)]
                vP = [pp.tile([128, NSK, 256], BF16, name=f"vP{j}")
                      for j in range(HG // 2)]

                ones = onp.tile([128, 128], BF16, tag="ones")
                # memset instead of DMA: available ~6us in, before the DMA
                # subsystem spins up (~9us), so warmups start immediately.
                nc.gpsimd.memset(ones[:], 1.0)

                # PE warmup: keep the array busy through the DMA lead-in so
                # the HAM clock gate reaches 8/8 before the first real MM.
                with tc.tile_pool(name="wup", bufs=1, space="PSUM") as wup:
                    wps = wup.tile([128, 128], F32, tag="wps")
                    for _ in range(120):
                        nc.tensor.matmul(wps[:], ones[:], ones[:],
                                         start=True, stop=True)

                # ---------------- Phase 1: QKV projection + RoPE ----------
                # Order: v(vc0) opens (its x first-touch rate ~287GB/s
                # fits under HBM, so x streams in stall-free), q/k follow,
                # v(vc1-3) closes phase 1 - giving an ~80us window where the
                # qk-side pools (csp/wp/evp) are idle so phase-2 pt/score
                # work can pre-fill before the transition.
                with (
                    tc.tile_pool(name="p1cs", bufs=1) as csp,
                    tc.tile_pool(name="p1xt", bufs=1) as xtp,
                    tc.tile_pool(name="p1w", bufs=2) as wp,
                    tc.tile_pool(name="p1vw", bufs=2) as vwp,
                    tc.tile_pool(name="p1ev", bufs=3) as evp,
                ):
                    xt = xtp.tile([128, NSB, NKT, SB], BF16, tag="xt")
                    # Everything rides the sync ring in need-time order.  The
                    # v-pass opens phase 1 because its x first-touch rate
                    # (~287GB/s) fits under HBM bandwidth, unlike the qk
                    # passes (~578GB/s) - so x streams in with no PE stalls.
                    nc.sync.dma_start(xt[:, 0, :, :], x_sb[0, :, :, :])
                    wv_first = vwp.tile([128, NKT, 256], BF16, tag="wv",
                                        name="wv_first")
                    nc.sync.dma_start(wv_first[:], w_v[0, :, :, :])
                    for sb in range(1, NSB):
                        nc.sync.dma_start(xt[:, sb, :, :], x_sb[sb, :, :, :])
                    cosf = csp.tile([128, S], BF16, tag="cos")
                    sinf = csp.tile([128, S], BF16, tag="sin")

                    def qk_pass(w_dram, dest, opener=False, hooks=None):
                        # An opener pass first touches only x chunk 0 (sb=0)
                        # for every head, giving the PE ~28us of work while
                        # the remaining x chunks load; hooks emit those chunk
                        # DMAs at the right sync-ring FIFO positions.
                        if opener:
                            rounds = [[0]] * HG + [[1, 2, 3]] * HG
                        else:
                            rounds = [[0, 1, 2, 3]] * HG
                        for i, sbs in enumerate(rounds):
                            c = i % HG
                            wc = wp.tile([128, NKT, 128], BF16, tag="w")
                            weng = nc.scalar if (opener and i == 0) else nc.sync
                            weng.dma_start(wc[:], w_dram[c, :, :, :])
                            for fn in (hooks or {}).get(i, ()):
                                fn()
                            for sb in sbs:
                                gsl = slice(sb * SB, (sb + 1) * SB)
                                ps = ps1.tile([128, SB], F32, tag="ps_qk")
                                for kt in range(NKT):
                                    nc.tensor.matmul(
                                        ps[:],
                                        wc[:, kt, :],
                                        xt[:, sb, kt, :],
                                        start=(kt == 0),
                                        stop=(kt == NKT - 1),
                                    )
                                qt = evp.tile([128, SB], BF16, tag="qt")
                                nc.scalar.copy(qt[:], ps[:])
                                qs = evp.tile([128, SB], BF16, tag="qs")
                                nc.gpsimd.dma_start(qs[0:64, :], qt[64:128, :])
                                nc.gpsimd.dma_start(qs[64:128, :], qt[0:64, :])
                                nc.vector.tensor_mul(qt[:], qt[:], cosf[:, gsl])
                                nc.vector.tensor_mul(qs[:], qs[:], sinf[:, gsl])
                                nc.vector.tensor_add(
                                    dest[c][:, gsl], qt[:], qs[:]
                                )

                    def v_pass(vcs):
                        for vc in vcs:
                            if vc == 0:
                                wvc = wv_first
                            else:
                                wvc = vwp.tile([128, NKT, 256], BF16,
                                               tag="wv")
                                nc.sync.dma_start(wvc[:], w_v[vc, :, :, :])
                            for ss in range(NSK):
                                sb, j = divmod(ss, 4)
                                ps = vps.tile([128, 256], F32, tag="ps_v")
                                for kt in range(NKT):
                                    nc.tensor.matmul(
                                        ps[:],
                                        xt[:, sb, kt,
                                           j * 128 : (j + 1) * 128],
                                        wvc[:, kt, :],
                                        start=(kt == 0),
                                        stop=(kt == NKT - 1),
                                    )
                                nc.vector.tensor_copy(vP[vc][:, ss, :], ps[:])

                    with (
                        tc.tile_pool(name="p1ps", bufs=4,
                                     space="PSUM") as ps1,
                        tc.tile_pool(name="p1vps", bufs=2,
                                     space="PSUM") as vps,
                    ):
                        v_pass([0])
                        nc.sync.dma_start(cosf[:], cos_in[:])
                        nc.sync.dma_start(sinf[:], sin_in[:])
                        qk_pass(w_q, qT)
                        qk_pass(w_k, kT)
                        v_pass([1, 2, 3])

                # ---------------- Phases 2+3 (fused, software-pipelined) --
                with (
                    tc.tile_pool(name="p2o", bufs=1) as otp,
                    tc.tile_pool(name="p3w", bufs=1) as wop,
                    tc.tile_pool(name="p2pt", bufs=10) as ptp,
                    tc.tile_pool(name="p2red", bufs=8) as rdp,
                    tc.tile_pool(name="p2r", bufs=1) as rp,
                    tc.tile_pool(name="p3o", bufs=4) as outp,
                    tc.tile_pool(name="p2ps_s", bufs=2, space="PSUM") as ps_s,
                    tc.tile_pool(name="p2ps_o", bufs=2, space="PSUM") as ps_o,
                    tc.tile_pool(name="p2ps_m", bufs=2, space="PSUM") as ps_m,
                ):
                    oT = [otp.tile([128, S], BF16, name=f"oT{h}")
                          for h in range(HG)]
                    wo = wop.tile([128, HG, HID], BF16, tag="wo")
                    nc.sync.dma_start(wo[:], w_o[:])

                    def emit_scores_exp_tree(h, sqb):
                        qsl = slice(sqb * SB, (sqb + 1) * SB)
                        pts = []
                        for sk2 in range(NSK // 2):
                            pss = ps_s.tile([128, 2 * SB], F32, tag="ps_s")
                            for j in range(2):
                                sk = 2 * sk2 + j
                                nc.tensor.matmul(
                                    pss[:, j * SB : (j + 1) * SB],
                                    kT[h][:, sk * 128 : (sk + 1) * 128],
                                    qT[h][:, qsl],
                                    start=True,
                                    stop=True,
                                )
                            pt = ptp.tile([128, 2 * SB], BF16, tag="pt")
                            nc.scalar.activation(pt[:], pss[:], EXP,
                                                 scale=SCALE)
                            pts.append(pt)
                        lvl = []
                        for i in range(8):
                            a = rdp.tile([128, SB], BF16, tag="ra")
                            nc.vector.tensor_add(
                                a[:], pts[i][:, 0:SB], pts[i][:, SB:2 * SB]
                            )
                            lvl.append(a)
                        while len(lvl) > 1:
                            nxt = []
                            for i in range(0, len(lvl), 2):
                                b = rdp.tile([128, SB], BF16, tag="rb")
                                nc.vector.tensor_add(b[:], lvl[i][:],
                                                     lvl[i + 1][:])
                                nxt.append(b)
                            lvl = nxt
                        return pts, lvl[0]

                    def emit_finish(h, sqb, pts, pacc):
                        qsl = slice(sqb * SB, (sqb + 1) * SB)
                        vp = vP[h // 2]
                        voff = (h % 2) * 128
                        pso = ps_o.tile([128, SB], F32, tag="ps_o")
                        for sk in range(NSK):
                            nc.tensor.matmul(
                                pso[:],
                                vp[:, sk, voff : voff + 128],
                                pts[sk // 2][:, (sk % 2) * SB
                                             : (sk % 2 + 1) * SB],
                                start=(sk == 0), stop=(sk == NSK - 1),
                            )
                        psd = ps_m.tile([128, SB], F32, tag="m")
                        nc.tensor.matmul(psd[:], ones[:], pacc[:],
                                         start=True, stop=True)
                        rec = rp.tile([128, SB], F32, tag="rec")
                        nc.vector.reciprocal_approx_fast(rec[:], psd[:])
                        nc.vector.tensor_mul(oT[h][:, qsl], pso[:], rec[:])

                    store_rings = [nc.sync, nc.gpsimd, nc.scalar]

                    def emit_p3(idx, sc, nb):
                        ssl = slice(sc * 128, (sc + 1) * 128)
                        ps = ps_m.tile([128, SB], F32, tag="m")
                        for h2 in range(HG):
                            nc.tensor.matmul(
                                ps[:],
                                oT[h2][:, ssl],
                                wo[:, h2, nb * SB : (nb + 1) * SB],
                                start=(h2 == 0),
                                stop=(h2 == HG - 1),
                            )
                        ot = outp.tile([128, SB], BF16, tag="out")
                        nc.vector.tensor_copy(ot[:], ps[:])
                        store_rings[idx % 3].dma_start(
                            out_p[ssl, nb * SB : (nb + 1) * SB], ot[:]
                        )

                    p3_idx = 0
                    prev = None
                    p3_pending = []
                    for sqb in range(NSB):
                        for h in range(HG):
                            cur = (h, sqb) + emit_scores_exp_tree(h, sqb)
                            if prev is not None:
                                emit_finish(*prev)
                                if prev[0] == HG - 1:
                                    r = prev[1]
                                    p3_pending.extend(
                                        (4 * r + i, nb)
                                        for i in range(4)
                                        for nb in range(HID // SB)
                                    )
                            for _ in range(2):
                                if p3_pending:
                                    emit_p3(p3_idx, *p3_pending.pop(0))
                                    p3_idx += 1
                            prev = cur
                    emit_finish(*prev)
                    p3_pending.extend(
                        (4 * (NSB - 1) + i, nb)
                        for i in range(4)
                        for nb in range(HID // SB)
                    )
                    for args in p3_pending:
                        emit_p3(p3_idx, *args)
                        p3_idx += 1

        nc.compile()
    finally:
        bacc_mod.get_activation_tables = orig_gat
    return nc


def _get_nc():
    if "nc" not in _STATE:
        _STATE["nc"] = _build()
    return _STATE["nc"]


def kernel(hidden_states, cos, sin, w_qkv, w_o):
    global LAST_RESULTS
    from concourse.bass_utils import run_bass_kernel_spmd
    import ml_dtypes

    bf16 = ml_dtypes.bfloat16

    trace = os.environ.get("KERNEL_TRACE", "") == "1"
    if trace:
        _ensure_ntff_hook()

    hidden_states = np.asarray(hidden_states, dtype=np.float32)
    cos = np.asarray(cos, dtype=np.float32)
    sin = np.asarray(sin, dtype=np.float32)
    w_qkv = np.asarray(w_qkv, dtype=np.float32)
    w_o = np.asarray(w_o, dtype=np.float32)

    cos_t = np.ascontiguousarray(cos.T)                      # [128, S]
    sin_t = np.ascontiguousarray(sin.T)
    sin_rot = np.concatenate([-sin_t[:64], sin_t[64:]], axis=0)

    def ktile(w):  # [HID, N] -> [128, NKT, N]
        n = w.shape[1]
        return np.ascontiguousarray(w.reshape(NKT, 128, n).transpose(1, 0, 2))

    def cblock(w, csz):  # [128, NKT, N] -> [N//csz, 128, NKT, csz]
        n = w.shape[2]
        return np.ascontiguousarray(
            w.reshape(128, NKT, n // csz, csz).transpose(2, 0, 1, 3)
        )

    in_maps = []
    for c in range(NCORES):
        b, g = divmod(c, 2)
        cs = slice(g * HG * D, (g + 1) * HG * D)
        wq = cblock(ktile(w_qkv[:, 0:H * D][:, cs]), 128)
        wk = cblock(ktile(w_qkv[:, H * D:2 * H * D][:, cs]), 128)
        wv = cblock(ktile(w_qkv[:, 2 * H * D:3 * H * D][:, cs]), 256)
        wo_c = w_o[cs, :]
        wo_r = np.ascontiguousarray(
            wo_c.reshape(HG, 128, HID).transpose(1, 0, 2)
        )
        # x^T k-tiled and s-blocked: [NSB, 128, NKT, SB]
        # x_sb[sb, p, kt, s] = x[sb*SB+s, kt*128+p]
        xt = hidden_states[b].T.reshape(NKT, 128, NSB, SB)
        xt = np.ascontiguousarray(xt.transpose(2, 1, 0, 3))
        in_maps.append({
            "x_sb": xt.astype(bf16),
            "w_q": wq.astype(bf16), "w_k": wk.astype(bf16),
            "w_v": wv.astype(bf16),
            "cos_in": cos_t.astype(bf16), "sin_in": sin_rot.astype(bf16),
            "w_o": wo_r.astype(bf16),
        })

    nc = _get_nc()
    res = run_bass_kernel_spmd(
        nc, in_maps, core_ids=list(range(NCORES)), trace=trace
    )
    LAST_RESULTS = res

    out = np.empty((B, S, HID), np.float32)
    for b in range(B):
        out[b] = (res.results[2 * b]["out_p"].astype(np.float32)
                  + res.results[2 * b + 1]["out_p"].astype(np.float32))
    return out
